# revision 13
# baseline (speedup 1.0000x reference)
"""Trainium2 Bass kernel for LocalDynamicGraph edge-feature construction.

Reference computation (per batch b, point n, neighbor slot k):
    out[b, n, c,      k] = x[b, idx[b,n,k], c] - x[b, n, c]   (c in [0,64))
    out[b, n, 64 + c, k] = x[b, n, c]
Output shape (B, N, 2C, K) = (8, 16384, 128, 20) float32.

Strategy: pure data parallel, one batch per NeuronCore (B == 8 cores).
Point n maps to partition p = n // 128, row i = n % 128, so every per-point
DRAM region a partition touches is contiguous:

  * x is loaded into SBUF once (32 KB/partition, 128 dense 32 KB
    descriptors); per-chunk center tiles are views into it.
  * Neighbor rows are fetched with SWDGE dma_gather (one 256 B descriptor
    per (n, k) pair) from HBM into an SBUF tile, partition = point.  Each
    chunk's gather is split into 1024-index sub-gathers (64 descriptors per
    SWDGE lane, single_packet=True) rotated across all 4 SWDGE queues: a
    sub-gather that fits the descriptor ring emits and retires in ~2.5 us
    instead of stalling the Q7 for the whole drain, which keeps the gather
    stream and the store stream interleaved on the 16 SDMA engines.
  * DVE computes (nbr - center) into a (128, J*2560) store tile whose free
    dim is exactly the DRAM layout of a point's (2C, K) block; ACT broadcasts
    the center half.  Stores are per-point-row 10 KB descriptors: short SDMA
    queue visits so gather rings never starve (40 KB descriptors regress).
  * idx tiles are prefetched in groups of 8 chunks on the Scalar HWDGE ring
    so they never queue behind dependent stores on the Sync sequencer.
"""

import sys

sys.path.insert(0, "/opt/trn_rl_repo")

import numpy as np

B, N, C, K = 8, 16384, 64, 20
P = 128          # SBUF partitions
J = 2            # points per partition per chunk
ROWS = N // P    # 128 point-rows per partition
PTS = P * J      # points per chunk
NIDX = PTS * K   # gather indices per chunk
IDX_COLS = NIDX // 16
M = 2 * C * K    # 2560 output elements per point
NCHUNK = ROWS // J

GATHER_SINGLE_PACKET = True
G_BUFS = 6
O_BUFS = 4
DMA_SCRATCH = 16384
SUB = 5
IDX_GROUP = 8            # sub-gathers per chunk (each fits the SWDGE ring)
SUB_IDX = NIDX // SUB
SUB_SLOTS = (J * K) // SUB
SUB_COLS = SUB_IDX // 16

_compiled = None


def _build(n_points: int):
    import concourse.bacc as bacc
    import concourse.mybir as mybir
    import concourse.tile as tile
    import concourse.bass as bass
    from concourse._compat import get_trn_type

    nchunk = (n_points // P) // J
    nc = bacc.Bacc(
        get_trn_type() or "TRN2",
        target_bir_lowering=False,
        debug=True,
        num_swdge_queues=4,
        dynamic_dma_scratch_size=DMA_SCRATCH,
    )
    x_in = nc.dram_tensor("x", [n_points, C], mybir.dt.float32, kind="ExternalInput")
    idx_in = nc.dram_tensor(
        "idxw", [P, nchunk * IDX_COLS], mybir.dt.int16, kind="ExternalInput"
    )
    y_out = nc.dram_tensor("y", [n_points, M], mybir.dt.float32, kind="ExternalOutput")

    # point n = p*ROWS + i  ->  partition p, row i; chunk u covers i in [uJ, uJ+J)
    x_v = x_in.rearrange("(p i) c -> p i c", p=P)
    y_v = y_out.rearrange("(p i) m -> p i m", p=P)

    with tile.TileContext(nc) as tc:
        with (
            tc.tile_pool(name="xp", bufs=1) as xp,
            tc.tile_pool(name="idxp", bufs=2 * IDX_GROUP) as idxp,
            tc.tile_pool(name="gp", bufs=G_BUFS) as gp,
            tc.tile_pool(name="op", bufs=O_BUFS) as op,
        ):
            x_sb = xp.tile([P, ROWS, C], mybir.dt.float32)
            nc.sync.dma_start(out=x_sb[:], in_=x_v)

            idx_tiles = {}

            def load_idx_group(base):
                for v in range(base, min(base + IDX_GROUP, nchunk)):
                    t = idxp.tile([P, IDX_COLS], mybir.dt.int16)
                    nc.scalar.dma_start(
                        out=t[:], in_=idx_in[:, v * IDX_COLS : (v + 1) * IDX_COLS]
                    )
                    idx_tiles[v] = t

            load_idx_group(0)
            for u in range(nchunk):
                if u % IDX_GROUP == 0:
                    load_idx_group(u + IDX_GROUP)
                idx_sb = idx_tiles.pop(u)
                g = gp.tile([P, J * K, C], mybir.dt.float32)
                for s in range(SUB):
                    nc.gpsimd.dma_gather(
                        g[:, s * SUB_SLOTS : (s + 1) * SUB_SLOTS, :],
                        x_in[:],
                        idx_sb[:, s * SUB_COLS : (s + 1) * SUB_COLS],
                        SUB_IDX,
                        SUB_IDX,
                        C,
                        single_packet=GATHER_SINGLE_PACKET,
                        queue_num=(u * SUB + s) % 4,
                    )

                o = op.tile([P, J, M], mybir.dt.float32)
                # (p, j, c, k) views of both halves of the store tile
                o_diff = o[:, :, : C * K].rearrange("p j (c k) -> p j c k", c=C)
                o_ctr = o[:, :, C * K :].rearrange("p j (c k) -> p j c k", c=C)
                g_vv = g[:].rearrange("p (j k) c -> p j c k", j=J)
                c_ap = x_sb[:, u * J : (u + 1) * J, :]
                ctr_bc = bass.AP(
                    c_ap.tensor, c_ap.offset, list(c_ap.ap) + [[0, K]]
                )
                nc.vector.tensor_sub(o_diff, g_vv, ctr_bc)
                nc.scalar.copy(o_ctr, ctr_bc)

                for j in range(J):
                    nc.sync.dma_start(out=y_v[:, u * J + j, :], in_=o[:, j, :])

    nc.compile()
    return nc


def _wrap_indices(idx_b: np.ndarray) -> np.ndarray:
    """idx_b (n_points, K) int -> (128, nchunk*IDX_COLS) int16 SBUF image.

    Gather-list position (j*K + k)*128 + p of chunk u holds the neighbor
    index idx[p*ROWS + u*J + j, k], wrapped 16-partition column-major and
    replicated to 128 partitions as the Q7 ucode expects.
    """
    a = idx_b.reshape(P, NCHUNK, J, K)        # [p, u, j, k]
    lin = a.transpose(1, 2, 3, 0).reshape(NCHUNK, NIDX)  # pos = (j*K+k)*128+p
    img = lin.reshape(NCHUNK, IDX_COLS, 16)   # (u, col, p16)
    img = img.transpose(2, 0, 1).reshape(16, -1)  # (16, nchunk*IDX_COLS)
    return np.tile(img, (8, 1)).astype(np.int16)


def kernel(x: np.ndarray, idx: np.ndarray) -> np.ndarray:
    from concourse.bass_utils import run_bass_kernel_spmd

    global _compiled
    if _compiled is None:
        _compiled = _build(N)
    nc = _compiled

    x = np.asarray(x, dtype=np.float32)
    idx = np.asarray(idx)
    in_maps = [
        {
            "x": np.ascontiguousarray(x[b]),
            "idxw": _wrap_indices(np.asarray(idx[b])),
        }
        for b in range(B)
    ]
    res = run_bass_kernel_spmd(nc, in_maps, core_ids=list(range(B)))
    out = np.stack([res.results[b]["y"].reshape(N, 2 * C, K) for b in range(B)])
    return out
